# revision 36
# baseline (speedup 1.0000x reference)
"""Bass/Tile Trainium2 kernel for nn_Attention (B=4, T=4096, C=256), 8 cores.

Sharding: core = (batch b, query-half h). Each core computes the full K/V
projections for its batch and attention output for its 2048 query rows.

Key-compaction: the 0/1 key mask is ~50% zeros, and masked keys contribute
nothing to the softmax (numerator or denominator). The host gathers the
valid keys of each batch into a compact array padded to a static VMAX
(multiple of 256), so the device processes ~18 key blocks instead of 32 —
roughly halving the score/exp/PV work. Padding keys have x=0, so k=v=0 and
their ones-column entry is 0: they drop out exactly like masked keys did.

Layout (all matmuls bf16, fp32 PSUM accumulation):
  - Host pre-transposes x to x^T [C, T]; projections contract C on
    partitions. k^T/q^T come out feature-major, so the score matmul
    produces scoresT [keys j on partitions, queries q on free dim].
  - Softmax needs no max-subtraction (exp of bf16 scores cannot overflow
    fp32) and no partition reductions.
  - V gets a column of ones appended (1 for real keys, 0 for padding):
    out[q, 256] accumulates the softmax denominator for free.
    Final: out[:, :256] * (1/out[:, 256]). The torch quirk (+1.0 bias on
    valid keys) cancels in softmax.
  - v-projection is interleaved into the first superblock's key loop so
    the score pipeline starts right after the q/k projections.
  - Main loop is software-pipelined per key block: PE does the two score
    matmuls for jb+1 and then the four out-matmuls for jb, so ACT's exp
    (~612 ns/tile) hides behind ~1.3 us of PE work.
"""

import contextlib

import numpy as np
import ml_dtypes

import concourse.bacc as bacc
import concourse.mybir as mybir
import concourse.tile as tile
from concourse.bass_utils import run_bass_kernel_spmd

B, T, C = 4, 4096, 256
NCORES = 8
HALVES = NCORES // B          # 2 query-halves per batch
TQ = T // HALVES              # 2048 query rows per core
PB = 128                      # partition block
NCCH = C // PB                # 2 contraction chunks of 128
SBW = 256                     # query superblock width
NSB = TQ // SBW               # 8 superblocks per core
NQB = SBW // PB               # 2 query 128-blocks per superblock
VW = C + 1                    # v tile width incl. ones column
SCALE = float(C) ** -0.5
BF16 = mybir.dt.bfloat16
F32 = mybir.dt.float32
DEF_VMAX = 2304               # default padded key count (18 blocks)


def _emit(tc, out, xt, xq, wq, wk, wv, mb, vmax, mode="full"):
    nc = tc.nc
    NJB = vmax // PB

    with contextlib.ExitStack() as ctx:
        persist = ctx.enter_context(tc.tile_pool(name="persist", bufs=1))
        # Persistent SBUF tensors; c-chunks laid side by side on the free dim.
        xt_sb = persist.tile([PB, NCCH * vmax], BF16)  # compacted x^T (keys)
        xq_sb = persist.tile([PB, NCCH * TQ], BF16)    # x^T (this core's half)
        wq_sb = persist.tile([PB, NCCH * C], BF16)
        wk_sb = persist.tile([PB, NCCH * C], BF16)
        wv_sb = persist.tile([PB, NCCH * C], BF16)
        kt_sb = persist.tile([PB, NCCH * vmax], BF16)  # k^T
        qt_sb = persist.tile([PB, NCCH * TQ], BF16)    # q^T
        va_sb = persist.tile([PB, NJB * VW], BF16)     # v + ones col
        mb_sb = persist.tile([PB, NJB], F32)           # key-valid 0/1

        # Few, large, descriptor-friendly DMAs spread across the three
        # DMA-capable queues. xq and weights land first so the q projection
        # starts while xt streams.
        w2 = lambda w: w.rearrange("(n p) c -> p n c", p=PB)
        s3 = lambda t, n: t.rearrange("p (n c) -> p n c", n=n)
        nc.scalar.dma_start(s3(wq_sb[:], NCCH), w2(wq))
        nc.scalar.dma_start(s3(wk_sb[:], NCCH), w2(wk))
        nc.gpsimd.dma_start(s3(wv_sb[:], NCCH), w2(wv))
        nc.gpsimd.dma_start(mb_sb[:], mb)
        if "noidma" in mode:
            nc.gpsimd.memset(xq_sb[:], 0.25)
            nc.gpsimd.memset(xt_sb[:], 0.25)
        else:
            nc.sync.dma_start(s3(xq_sb[:], NCCH),
                              xq.rearrange("(n p) t -> p n t", p=PB))
            H = vmax // 2
            nc.sync.dma_start(xt_sb[:, 0:H], xt[0:PB, 0:H])
            nc.scalar.dma_start(xt_sb[:, vmax:vmax + H], xt[PB:2 * PB, 0:H])
            nc.sync.dma_start(xt_sb[:, H:vmax], xt[0:PB, H:vmax])
            nc.scalar.dma_start(xt_sb[:, vmax + H:2 * vmax], xt[PB:2 * PB, H:vmax])

        # ones column: va[:, jb*VW + C] = key-valid[:, jb]
        va_ones = va_sb[:].rearrange("p (j e) -> p j e", e=VW)[:, :, C:C + 1]
        nc.vector.tensor_copy(va_ones, mb_sb[:].rearrange("p (j e) -> p j e", e=1))

        # ---- q/k projections ----
        with tc.tile_pool(name="proj_psum", bufs=2, space="PSUM") as pp:
            # q^T[d, t] / k^T[d, t]: lhsT = W^T chunk [c, d], rhs = x^T [c, t]
            for w_sb, x_src, x_w, dst, copy_eng in (
                (wq_sb, xq_sb, TQ, qt_sb, nc.vector.tensor_copy),
                (wk_sb, xt_sb, vmax, kt_sb, nc.vector.tensor_copy),
            ):
                for s0 in range(0, x_w, 512):
                    sw = min(512, x_w - s0)
                    for dc in range(NCCH):
                        ps = pp.tile([PB, 512], F32, tag="proj", name="proj_ps")
                        for cc in range(NCCH):
                            nc.tensor.matmul(
                                ps[:, 0:sw],
                                lhsT=w_sb[:, cc * C + dc * PB: cc * C + (dc + 1) * PB],
                                rhs=x_src[:, cc * x_w + s0: cc * x_w + s0 + sw],
                                start=(cc == 0),
                                stop=(cc == NCCH - 1),
                            )
                        copy_eng(dst[:, dc * x_w + s0: dc * x_w + s0 + sw],
                                 ps[:, 0:sw])

        # ---- attention main loop (v-projection interleaved into sb 0) ----
        # Key blocks are processed in PAIRS: both halves of a [PB, 2*SBW]
        # PSUM tile get score matmuls, then ONE wide exp covers the pair —
        # the ACT fixed per-instruction cost (~270 ns on HW) halves.
        # The vproj tiles share the sc ring so PSUM stays within 8 banks:
        # scp 3x2 + op 2 = 8.
        NJP = (NJB + 1) // 2
        scb, opb = (2, 2) if "opdouble" in mode else (3, 1)
        scp = ctx.enter_context(tc.tile_pool(name="sc_psum", bufs=scb, space="PSUM"))
        op = ctx.enter_context(tc.tile_pool(name="o_psum", bufs=opb, space="PSUM"))
        stage = ctx.enter_context(tc.tile_pool(name="stage", bufs=5))
        ppool = ctx.enter_context(tc.tile_pool(name="p_pool", bufs=6))
        fin = ctx.enter_context(tc.tile_pool(name="fin", bufs=3))

        if mode == "noscores":
            p_static = persist.tile([PB, 4 * SBW], BF16, name="p_static")
            nc.vector.memset(p_static[:], 1.0)

        assert SBW == C

        def emit_vproj(jp):
            # v[t, d] for key-block pair jp: lhsT = x^T chunk [c, t-block],
            # rhs = W^T chunk [c, d]; both blocks land in one sc-ring tile
            # and move to va with a single strided copy.
            jbs = [j for j in (2 * jp, 2 * jp + 1) if j < NJB]
            ps = scp.tile([PB, 2 * SBW], F32, tag="sc", name="v_ps")
            for i, jb in enumerate(jbs):
                for cc in range(NCCH):
                    nc.tensor.matmul(
                        ps[:, i * C:(i + 1) * C],
                        lhsT=xt_sb[:, cc * vmax + jb * PB: cc * vmax + (jb + 1) * PB],
                        rhs=wv_sb[:, cc * C:(cc + 1) * C],
                        start=(cc == 0),
                        stop=(cc == NCCH - 1),
                    )
            va3 = va_sb[:].rearrange("p (j e) -> p j e", e=VW)
            nc.vector.tensor_copy(
                va3[:, 2 * jp:2 * jp + len(jbs), 0:C],
                ps[:, 0:len(jbs) * C].rearrange("p (n c) -> p n c", c=C))

        for sb in range(NSB):
            if mode == "noout":
                op_tiles = None
            else:
                op_tiles = [op.tile([PB, VW], F32, tag=f"o{qb}", name=f"opsum{qb}")
                            for qb in range(NQB)]
            p_tiles = {}

            def emit_scores(jp, sb=sb, p_tiles=p_tiles):
                # scores for key-block pair jp into one [PB, 2*SBW] PSUM
                # tile. The raw scores are staged to SBUF by the (mostly
                # idle) DVE so the PSUM slot frees after a ~600 ns copy
                # instead of after exp — the exp+PV chain then pipelines
                # deeply in SBUF, beyond the 8-bank PSUM limit.
                jbs = [j for j in (2 * jp, 2 * jp + 1) if j < NJB]
                w = len(jbs) * SBW
                ps = scp.tile([PB, 2 * SBW], F32, tag="sc", name="sc_ps")
                for i, jb in enumerate(jbs):
                    for cc in range(NCCH):
                        nc.tensor.matmul(
                            ps[:, i * SBW:(i + 1) * SBW],
                            lhsT=kt_sb[:, cc * vmax + jb * PB: cc * vmax + (jb + 1) * PB],
                            rhs=qt_sb[:, cc * TQ + sb * SBW: cc * TQ + (sb + 1) * SBW],
                            start=(cc == 0),
                            stop=(cc == NCCH - 1),
                        )
                st = stage.tile([PB, 2 * SBW], F32, tag="st", name="st_t")
                nc.vector.tensor_copy(st[:, 0:w], ps[:, 0:w])
                pt = ppool.tile([PB, 2 * SBW], BF16, tag="p", name="p_t")
                nc.scalar.activation(
                    pt[:, 0:w], st[:, 0:w],
                    mybir.ActivationFunctionType.Exp, scale=SCALE)
                p_tiles[jp] = pt

            def emit_out(jb, op_tiles=op_tiles, p_tiles=p_tiles):
                pt = p_tiles[jb // 2]
                if jb % 2 == 1 or jb == NJB - 1:
                    del p_tiles[jb // 2]
                for qb in range(NQB):
                    nc.tensor.matmul(
                        op_tiles[qb],
                        lhsT=pt[:, (jb % 2) * SBW + qb * PB:
                                (jb % 2) * SBW + (qb + 1) * PB],
                        rhs=va_sb[:, jb * VW:(jb + 1) * VW],
                        start=(jb == 0),
                        stop=(jb == NJB - 1),
                    )

            if mode == "noout":
                for jp in range(NJP):
                    if sb == 0:
                        emit_vproj(jp)
                    emit_scores(jp)
                    p_tiles.pop(jp)
            elif mode == "noscores":
                if sb == 0:
                    for jp in range(NJP):
                        emit_vproj(jp)
                for jb in range(NJB):
                    for qb in range(NQB):
                        nc.tensor.matmul(
                            op_tiles[qb],
                            lhsT=p_static[:, (jb % 4) * SBW + qb * PB:
                                          (jb % 4) * SBW + (qb + 1) * PB],
                            rhs=va_sb[:, jb * VW:(jb + 1) * VW],
                            start=(jb == 0),
                            stop=(jb == NJB - 1),
                        )
            elif sb == 0:
                # sb 0 carries the v projection, interleaved into the pair
                # loop; the shared sc ring turns over quickly now that both
                # scores and vproj free their PSUM slot via a DVE copy.
                for p in range(min(3, NJP)):
                    emit_vproj(p)
                for p in range(min(2, NJP)):
                    emit_scores(p)
                for jp in range(NJP):
                    if jp + 2 < NJP:
                        emit_scores(jp + 2)
                    if jp + 3 < NJP:
                        emit_vproj(jp + 3)
                    for jb in (2 * jp, 2 * jp + 1):
                        if jb < NJB:
                            emit_out(jb)
            else:
                # pair-lookahead 4: the staging copy + wide exp (~1.6 us
                # with semaphore hops on HW) hides behind ~3 us of PE work.
                for p in range(min(4, NJP)):
                    emit_scores(p)
                for jp in range(NJP):
                    if jp + 4 < NJP:
                        emit_scores(jp + 4)
                    for jb in (2 * jp, 2 * jp + 1):
                        if jb < NJB:
                            emit_out(jb)
            if mode == "noout":
                os_t = fin.tile([PB, C], BF16, tag="os", name="os_t")
                nc.vector.tensor_copy(os_t, kt_sb[:, sb * C:(sb + 1) * C])
                nc.sync.dma_start(out[sb * PB:(sb + 1) * PB, :], os_t)
                continue
            if "notail" in mode:
                if sb == NSB - 1:
                    os_t = fin.tile([PB, C], BF16, tag="os", name="os_t")
                    nc.vector.tensor_copy(os_t, op_tiles[0][:, 0:C])
                    nc.sync.dma_start(out[0:PB, :], os_t)
                continue
            os_t = fin.tile([PB, NQB * C], BF16, tag="os", name="os_t")
            for qb in range(NQB):
                rec = fin.tile([PB, 1], F32, tag="rec", name="rec_t")
                nc.vector.reciprocal(rec, op_tiles[qb][:, C:C + 1])
                nc.vector.tensor_scalar_mul(
                    os_t[:, qb * C:(qb + 1) * C], op_tiles[qb][:, 0:C], rec)
            dma_eng = nc.sync if sb % 2 == 0 else nc.scalar
            dma_eng.dma_start(
                out[sb * SBW:(sb + 1) * SBW, :].rearrange("(q p) c -> p q c", p=PB),
                os_t[:].rearrange("p (q c) -> p q c", q=NQB))


def build_nc(reps=1, loop_n=0, mode="full", vmax=None):
    if vmax is None:
        vmax = _CACHE.get("vmax", DEF_VMAX)
    nc = bacc.Bacc("TRN2", target_bir_lowering=False, debug=False)
    xt = nc.dram_tensor("xt", [C, vmax], BF16, kind="ExternalInput").ap()
    xq = nc.dram_tensor("xq", [C, TQ], BF16, kind="ExternalInput").ap()
    wq = nc.dram_tensor("wq", [C, C], BF16, kind="ExternalInput").ap()
    wk = nc.dram_tensor("wk", [C, C], BF16, kind="ExternalInput").ap()
    wv = nc.dram_tensor("wv", [C, C], BF16, kind="ExternalInput").ap()
    mb = nc.dram_tensor("mb", [PB, vmax // PB], F32, kind="ExternalInput").ap()
    out = nc.dram_tensor("out", [TQ, C], BF16, kind="ExternalOutput").ap()
    with tile.TileContext(nc) as tc:
        if loop_n:
            # For_i carries an InstAllEngineBarrier per iteration (sem
            # reset), which drains every engine + DMA queue and costs
            # ~25-30 us on HW. Unroll several kernel bodies per iteration
            # so the barrier amortizes; bodies overlap via normal tile
            # WAR deps.
            unroll = next(u for u in (16, 8, 4, 2, 1) if loop_n % u == 0)
            with tc.For_i(0, loop_n // unroll, 1,
                          hint_engines=(mybir.EngineType.PE,)):
                for _ in range(unroll):
                    _emit(tc, out, xt, xq, wq, wk, wv, mb, vmax, mode=mode)
        else:
            for _ in range(reps):
                _emit(tc, out, xt, xq, wq, wk, wv, mb, vmax, mode=mode)
    nc.compile()
    return nc


_CACHE = {}


def _get_nc():
    vmax = _CACHE.get("vmax", DEF_VMAX)
    key = ("nc", vmax)
    if key not in _CACHE:
        _CACHE[key] = build_nc(vmax=vmax)
    return _CACHE[key]


def make_in_maps(x, mask):
    bf = ml_dtypes.bfloat16
    x = np.asarray(x, dtype=np.float32)
    m01 = np.asarray(mask) != 0                    # [B, T]
    nvalid = m01.sum(axis=1)
    vmax = max(256, int(-(-int(nvalid.max()) // 128) * 128))
    _CACHE["vmax"] = vmax
    njb = vmax // PB
    xt_all = x.transpose(0, 2, 1)                  # [B, C, T] fp32
    xc_b, mb_b = [], []
    for b in range(B):
        idx = np.nonzero(m01[b])[0]
        xc = np.zeros((C, vmax), np.float32)
        xc[:, :len(idx)] = xt_all[b][:, idx]
        xc_b.append(xc.astype(bf))
        mbc = (np.arange(vmax) < len(idx)).astype(np.float32).reshape(njb, PB).T
        mb_b.append(np.ascontiguousarray(mbc))
    maps = []
    for core in range(NCORES):
        b, h = divmod(core, HALVES)
        maps.append({
            "xt": xc_b[b],
            "xq": np.ascontiguousarray(xt_all[b][:, h * TQ:(h + 1) * TQ]).astype(bf),
            "mb": mb_b[b],
        })
    return maps


def kernel(x, mask, Wk, Wq, Wv):
    bf = ml_dtypes.bfloat16
    wqt = np.ascontiguousarray(np.asarray(Wq, dtype=np.float32).T).astype(bf)
    wkt = np.ascontiguousarray(np.asarray(Wk, dtype=np.float32).T).astype(bf)
    wvt = np.ascontiguousarray(np.asarray(Wv, dtype=np.float32).T).astype(bf)
    in_maps = make_in_maps(x, mask)
    for m in in_maps:
        m.update({"wq": wqt, "wk": wkt, "wv": wvt})
    res = run_bass_kernel_spmd(_get_nc(), in_maps, list(range(NCORES)))
    out = np.empty((B, T, C), np.float32)
    for core in range(NCORES):
        b, h = divmod(core, HALVES)
        out[b, h * TQ:(h + 1) * TQ, :] = np.asarray(
            res.results[core]["out"], dtype=np.float32)
    return out


# revision 37
# speedup vs baseline: 1.3468x; 1.3468x over previous
"""Bass/Tile Trainium2 kernel for nn_Attention (B=4, T=4096, C=256), 8 cores.

Sharding: core = (batch b, query-half h). Each core computes the full K/V
projections for its batch and attention output for its 2048 query rows.

Key-compaction: the 0/1 key mask is ~50% zeros, and masked keys contribute
nothing to the softmax (numerator or denominator). The host gathers the
valid keys of each batch into a compact array padded to a static VMAX
(multiple of 256), so the device processes ~18 key blocks instead of 32 —
roughly halving the score/exp/PV work. Padding keys have x=0, so k=v=0 and
their ones-column entry is 0: they drop out exactly like masked keys did.

Layout (all matmuls bf16, fp32 PSUM accumulation):
  - Host pre-transposes x to x^T [C, T]; projections contract C on
    partitions. k^T/q^T come out feature-major, so the score matmul
    produces scoresT [keys j on partitions, queries q on free dim].
  - Softmax needs no max-subtraction (exp of bf16 scores cannot overflow
    fp32) and no partition reductions.
  - V gets a column of ones appended (1 for real keys, 0 for padding):
    out[q, 256] accumulates the softmax denominator for free.
    Final: out[:, :256] * (1/out[:, 256]). The torch quirk (+1.0 bias on
    valid keys) cancels in softmax.
  - v-projection is interleaved into the first superblock's key loop so
    the score pipeline starts right after the q/k projections.
  - Main loop is software-pipelined per key block: PE does the two score
    matmuls for jb+1 and then the four out-matmuls for jb, so ACT's exp
    (~612 ns/tile) hides behind ~1.3 us of PE work.
"""

import contextlib

import numpy as np
import ml_dtypes

import concourse.bacc as bacc
import concourse.mybir as mybir
import concourse.tile as tile
from concourse.bass_utils import run_bass_kernel_spmd

B, T, C = 4, 4096, 256
NCORES = 8
HALVES = NCORES // B          # 2 query-halves per batch
TQ = T // HALVES              # 2048 query rows per core
PB = 128                      # partition block
NCCH = C // PB                # 2 contraction chunks of 128
SBW = 256                     # query superblock width
NSB = TQ // SBW               # 8 superblocks per core
NQB = SBW // PB               # 2 query 128-blocks per superblock
VW = C + 1                    # v tile width incl. ones column
SCALE = float(C) ** -0.5
BF16 = mybir.dt.bfloat16
F32 = mybir.dt.float32
DEF_VMAX = 2304               # default padded key count (18 blocks)


def _emit(tc, out, xt, xq, wq, wk, wv, mb, vmax, mode="full"):
    nc = tc.nc
    NJB = vmax // PB

    with contextlib.ExitStack() as ctx:
        persist = ctx.enter_context(tc.tile_pool(name="persist", bufs=1))
        # Persistent SBUF tensors; c-chunks laid side by side on the free dim.
        xt_sb = persist.tile([PB, NCCH * vmax], BF16)  # compacted x^T (keys)
        xq_sb = persist.tile([PB, NCCH * TQ], BF16)    # x^T (this core's half)
        wq_sb = persist.tile([PB, NCCH * C], BF16)
        wk_sb = persist.tile([PB, NCCH * C], BF16)
        wv_sb = persist.tile([PB, NCCH * C], BF16)
        kt_sb = persist.tile([PB, NCCH * vmax], BF16)  # k^T
        qt_sb = persist.tile([PB, NCCH * TQ], BF16)    # q^T
        va_sb = persist.tile([PB, NJB * VW], BF16)     # v + ones col
        mb_sb = persist.tile([PB, NJB], F32)           # key-valid 0/1

        # Few, large, descriptor-friendly DMAs spread across the three
        # DMA-capable queues. xq and weights land first so the q projection
        # starts while xt streams.
        w2 = lambda w: w.rearrange("(n p) c -> p n c", p=PB)
        s3 = lambda t, n: t.rearrange("p (n c) -> p n c", n=n)
        nc.scalar.dma_start(s3(wq_sb[:], NCCH), w2(wq))
        nc.scalar.dma_start(s3(wk_sb[:], NCCH), w2(wk))
        nc.gpsimd.dma_start(s3(wv_sb[:], NCCH), w2(wv))
        nc.gpsimd.dma_start(mb_sb[:], mb)
        if "noidma" in mode:
            nc.gpsimd.memset(xq_sb[:], 0.25)
            nc.gpsimd.memset(xt_sb[:], 0.25)
        else:
            nc.sync.dma_start(s3(xq_sb[:], NCCH),
                              xq.rearrange("(n p) t -> p n t", p=PB))
            H = vmax // 2
            nc.sync.dma_start(xt_sb[:, 0:H], xt[0:PB, 0:H])
            nc.scalar.dma_start(xt_sb[:, vmax:vmax + H], xt[PB:2 * PB, 0:H])
            nc.sync.dma_start(xt_sb[:, H:vmax], xt[0:PB, H:vmax])
            nc.scalar.dma_start(xt_sb[:, vmax + H:2 * vmax], xt[PB:2 * PB, H:vmax])

        # ones column: va[:, jb*VW + C] = key-valid[:, jb]
        va_ones = va_sb[:].rearrange("p (j e) -> p j e", e=VW)[:, :, C:C + 1]
        nc.vector.tensor_copy(va_ones, mb_sb[:].rearrange("p (j e) -> p j e", e=1))

        # ---- q/k projections ----
        with tc.tile_pool(name="proj_psum", bufs=2, space="PSUM") as pp:
            # q^T[d, t] / k^T[d, t]: lhsT = W^T chunk [c, d], rhs = x^T [c, t]
            for w_sb, x_src, x_w, dst, copy_eng in (
                (wq_sb, xq_sb, TQ, qt_sb, nc.vector.tensor_copy),
                (wk_sb, xt_sb, vmax, kt_sb, nc.vector.tensor_copy),
            ):
                for s0 in range(0, x_w, 512):
                    sw = min(512, x_w - s0)
                    for dc in range(NCCH):
                        ps = pp.tile([PB, 512], F32, tag="proj", name="proj_ps")
                        for cc in range(NCCH):
                            nc.tensor.matmul(
                                ps[:, 0:sw],
                                lhsT=w_sb[:, cc * C + dc * PB: cc * C + (dc + 1) * PB],
                                rhs=x_src[:, cc * x_w + s0: cc * x_w + s0 + sw],
                                start=(cc == 0),
                                stop=(cc == NCCH - 1),
                            )
                        copy_eng(dst[:, dc * x_w + s0: dc * x_w + s0 + sw],
                                 ps[:, 0:sw])

        # ---- attention main loop (v-projection interleaved into sb 0) ----
        # Key blocks are processed in PAIRS: both halves of a [PB, 2*SBW]
        # PSUM tile get score matmuls, then ONE wide exp covers the pair —
        # the ACT fixed per-instruction cost (~270 ns on HW) halves.
        # The vproj tiles share the sc ring so PSUM stays within 8 banks:
        # scp 3x2 + op 2 = 8.
        NJP = (NJB + 1) // 2
        scb, opb = (2, 2) if "opdouble" in mode else (3, 1)
        scp = ctx.enter_context(tc.tile_pool(name="sc_psum", bufs=scb, space="PSUM"))
        op = ctx.enter_context(tc.tile_pool(name="o_psum", bufs=opb, space="PSUM"))
        stage = ctx.enter_context(tc.tile_pool(name="stage", bufs=5))
        ppool = ctx.enter_context(tc.tile_pool(name="p_pool", bufs=6))
        fin = ctx.enter_context(tc.tile_pool(name="fin", bufs=3))

        if mode == "noscores":
            p_static = persist.tile([PB, 4 * SBW], BF16, name="p_static")
            nc.vector.memset(p_static[:], 1.0)

        assert SBW == C

        def emit_vproj(jp):
            # v[t, d] for key-block pair jp: lhsT = x^T chunk [c, t-block],
            # rhs = W^T chunk [c, d]; both blocks land in one sc-ring tile
            # and move to va with a single strided copy.
            jbs = [j for j in (2 * jp, 2 * jp + 1) if j < NJB]
            ps = scp.tile([PB, 2 * SBW], F32, tag="sc", name="v_ps")
            for i, jb in enumerate(jbs):
                for cc in range(NCCH):
                    nc.tensor.matmul(
                        ps[:, i * C:(i + 1) * C],
                        lhsT=xt_sb[:, cc * vmax + jb * PB: cc * vmax + (jb + 1) * PB],
                        rhs=wv_sb[:, cc * C:(cc + 1) * C],
                        start=(cc == 0),
                        stop=(cc == NCCH - 1),
                    )
            va3 = va_sb[:].rearrange("p (j e) -> p j e", e=VW)
            nc.vector.tensor_copy(
                va3[:, 2 * jp:2 * jp + len(jbs), 0:C],
                ps[:, 0:len(jbs) * C].rearrange("p (n c) -> p n c", c=C))

        for sb in range(NSB):
            if mode == "noout":
                op_tiles = None
            else:
                op_tiles = [op.tile([PB, VW], F32, tag=f"o{qb}", name=f"opsum{qb}")
                            for qb in range(NQB)]
            p_tiles = {}

            def emit_scores(jp, sb=sb, p_tiles=p_tiles):
                # scores for key-block pair jp into one [PB, 2*SBW] PSUM
                # tile. The raw scores are staged to SBUF by the (mostly
                # idle) DVE so the PSUM slot frees after a ~600 ns copy
                # instead of after exp — the exp+PV chain then pipelines
                # deeply in SBUF, beyond the 8-bank PSUM limit.
                jbs = [j for j in (2 * jp, 2 * jp + 1) if j < NJB]
                w = len(jbs) * SBW
                ps = scp.tile([PB, 2 * SBW], F32, tag="sc", name="sc_ps")
                for i, jb in enumerate(jbs):
                    for cc in range(NCCH):
                        nc.tensor.matmul(
                            ps[:, i * SBW:(i + 1) * SBW],
                            lhsT=kt_sb[:, cc * vmax + jb * PB: cc * vmax + (jb + 1) * PB],
                            rhs=qt_sb[:, cc * TQ + sb * SBW: cc * TQ + (sb + 1) * SBW],
                            start=(cc == 0),
                            stop=(cc == NCCH - 1),
                        )
                st = stage.tile([PB, 2 * SBW], F32, tag="st", name="st_t")
                nc.vector.tensor_copy(st[:, 0:w], ps[:, 0:w])
                pt = ppool.tile([PB, 2 * SBW], BF16, tag="p", name="p_t")
                nc.scalar.activation(
                    pt[:, 0:w], st[:, 0:w],
                    mybir.ActivationFunctionType.Exp, scale=SCALE)
                p_tiles[jp] = pt

            def emit_out(jb, op_tiles=op_tiles, p_tiles=p_tiles):
                pt = p_tiles[jb // 2]
                if jb % 2 == 1 or jb == NJB - 1:
                    del p_tiles[jb // 2]
                for qb in range(NQB):
                    nc.tensor.matmul(
                        op_tiles[qb],
                        lhsT=pt[:, (jb % 2) * SBW + qb * PB:
                                (jb % 2) * SBW + (qb + 1) * PB],
                        rhs=va_sb[:, jb * VW:(jb + 1) * VW],
                        start=(jb == 0),
                        stop=(jb == NJB - 1),
                    )

            if mode == "noout":
                for jp in range(NJP):
                    if sb == 0:
                        emit_vproj(jp)
                    emit_scores(jp)
                    p_tiles.pop(jp)
            elif mode == "noscores":
                if sb == 0:
                    for jp in range(NJP):
                        emit_vproj(jp)
                for jb in range(NJB):
                    for qb in range(NQB):
                        nc.tensor.matmul(
                            op_tiles[qb],
                            lhsT=p_static[:, (jb % 4) * SBW + qb * PB:
                                          (jb % 4) * SBW + (qb + 1) * PB],
                            rhs=va_sb[:, jb * VW:(jb + 1) * VW],
                            start=(jb == 0),
                            stop=(jb == NJB - 1),
                        )
            elif sb == 0:
                # sb 0 carries the v projection, interleaved into the pair
                # loop; the shared sc ring turns over quickly now that both
                # scores and vproj free their PSUM slot via a DVE copy.
                for p in range(min(3, NJP)):
                    emit_vproj(p)
                for p in range(min(2, NJP)):
                    emit_scores(p)
                for jp in range(NJP):
                    if jp + 2 < NJP:
                        emit_scores(jp + 2)
                    if jp + 3 < NJP:
                        emit_vproj(jp + 3)
                    for jb in (2 * jp, 2 * jp + 1):
                        if jb < NJB:
                            emit_out(jb)
            else:
                # pair-lookahead 4: the staging copy + wide exp (~1.6 us
                # with semaphore hops on HW) hides behind ~3 us of PE work.
                for p in range(min(4, NJP)):
                    emit_scores(p)
                for jp in range(NJP):
                    if jp + 4 < NJP:
                        emit_scores(jp + 4)
                    for jb in (2 * jp, 2 * jp + 1):
                        if jb < NJB:
                            emit_out(jb)
            if mode == "noout":
                os_t = fin.tile([PB, C], BF16, tag="os", name="os_t")
                nc.vector.tensor_copy(os_t, kt_sb[:, sb * C:(sb + 1) * C])
                nc.sync.dma_start(out[sb * PB:(sb + 1) * PB, :], os_t)
                continue
            if "notail" in mode:
                if sb == NSB - 1:
                    os_t = fin.tile([PB, C], BF16, tag="os", name="os_t")
                    nc.vector.tensor_copy(os_t, op_tiles[0][:, 0:C])
                    nc.sync.dma_start(out[0:PB, :], os_t)
                continue
            os_t = fin.tile([PB, NQB * C], BF16, tag="os", name="os_t")
            for qb in range(NQB):
                rec = fin.tile([PB, 1], F32, tag="rec", name="rec_t")
                nc.vector.reciprocal(rec, op_tiles[qb][:, C:C + 1])
                nc.vector.tensor_scalar_mul(
                    os_t[:, qb * C:(qb + 1) * C], op_tiles[qb][:, 0:C], rec)
            dma_eng = nc.sync if sb % 2 == 0 else nc.scalar
            dma_eng.dma_start(
                out[sb * SBW:(sb + 1) * SBW, :].rearrange("(q p) c -> p q c", p=PB),
                os_t[:].rearrange("p (q c) -> p q c", q=NQB))


def build_nc(reps=1, loop_n=0, mode="full", vmax=None):
    if vmax is None:
        vmax = _CACHE.get("vmax", DEF_VMAX)
    nc = bacc.Bacc("TRN2", target_bir_lowering=False, debug=False)
    xt = nc.dram_tensor("xt", [C, vmax], BF16, kind="ExternalInput").ap()
    xq = nc.dram_tensor("xq", [C, TQ], BF16, kind="ExternalInput").ap()
    wq = nc.dram_tensor("wq", [C, C], BF16, kind="ExternalInput").ap()
    wk = nc.dram_tensor("wk", [C, C], BF16, kind="ExternalInput").ap()
    wv = nc.dram_tensor("wv", [C, C], BF16, kind="ExternalInput").ap()
    mb = nc.dram_tensor("mb", [PB, vmax // PB], F32, kind="ExternalInput").ap()
    out = nc.dram_tensor("out", [TQ, C], BF16, kind="ExternalOutput").ap()
    with tile.TileContext(nc) as tc:
        if loop_n:
            # For_i carries an InstAllEngineBarrier per iteration (sem
            # reset), which drains every engine + DMA queue and costs
            # ~25-30 us on HW. Unroll several kernel bodies per iteration
            # so the barrier amortizes; bodies overlap via normal tile
            # WAR deps.
            # unroll=8 amortizes the barrier; 16 measured WORSE (155 us vs
            # ~110) — the ~9k-instruction body likely exceeds sequencer
            # instruction-fetch capacity, while ~4.6k still fits.
            unroll = next(u for u in (8, 4, 2, 1) if loop_n % u == 0)
            with tc.For_i(0, loop_n // unroll, 1,
                          hint_engines=(mybir.EngineType.PE,)):
                for _ in range(unroll):
                    _emit(tc, out, xt, xq, wq, wk, wv, mb, vmax, mode=mode)
        else:
            for _ in range(reps):
                _emit(tc, out, xt, xq, wq, wk, wv, mb, vmax, mode=mode)
    nc.compile()
    return nc


_CACHE = {}


def _get_nc():
    vmax = _CACHE.get("vmax", DEF_VMAX)
    key = ("nc", vmax)
    if key not in _CACHE:
        _CACHE[key] = build_nc(vmax=vmax)
    return _CACHE[key]


def make_in_maps(x, mask):
    bf = ml_dtypes.bfloat16
    x = np.asarray(x, dtype=np.float32)
    m01 = np.asarray(mask) != 0                    # [B, T]
    nvalid = m01.sum(axis=1)
    vmax = max(256, int(-(-int(nvalid.max()) // 128) * 128))
    _CACHE["vmax"] = vmax
    njb = vmax // PB
    xt_all = x.transpose(0, 2, 1)                  # [B, C, T] fp32
    xc_b, mb_b = [], []
    for b in range(B):
        idx = np.nonzero(m01[b])[0]
        xc = np.zeros((C, vmax), np.float32)
        xc[:, :len(idx)] = xt_all[b][:, idx]
        xc_b.append(xc.astype(bf))
        mbc = (np.arange(vmax) < len(idx)).astype(np.float32).reshape(njb, PB).T
        mb_b.append(np.ascontiguousarray(mbc))
    maps = []
    for core in range(NCORES):
        b, h = divmod(core, HALVES)
        maps.append({
            "xt": xc_b[b],
            "xq": np.ascontiguousarray(xt_all[b][:, h * TQ:(h + 1) * TQ]).astype(bf),
            "mb": mb_b[b],
        })
    return maps


def kernel(x, mask, Wk, Wq, Wv):
    bf = ml_dtypes.bfloat16
    wqt = np.ascontiguousarray(np.asarray(Wq, dtype=np.float32).T).astype(bf)
    wkt = np.ascontiguousarray(np.asarray(Wk, dtype=np.float32).T).astype(bf)
    wvt = np.ascontiguousarray(np.asarray(Wv, dtype=np.float32).T).astype(bf)
    in_maps = make_in_maps(x, mask)
    for m in in_maps:
        m.update({"wq": wqt, "wk": wkt, "wv": wvt})
    res = run_bass_kernel_spmd(_get_nc(), in_maps, list(range(NCORES)))
    out = np.empty((B, T, C), np.float32)
    for core in range(NCORES):
        b, h = divmod(core, HALVES)
        out[b, h * TQ:(h + 1) * TQ, :] = np.asarray(
            res.results[core]["out"], dtype=np.float32)
    return out
